# revision 3
# baseline (speedup 1.0000x reference)
"""Pixel-RNN control kernel for TRN2 (8 NeuronCores, batch-sharded).

Math: h_t = tanh(x_t @ W_ih^T + h_{t-1} @ W_hh^T + b), out_t = sigmoid(10*h_t).
With W_hh = I, b = 0, W_ih = diag(s), |s_c| = 1 (the harness fixes
s = (-1, +1)), substitute g = s*h (s^2 = 1, tanh odd):
    g_t = tanh(x_t + g_{t-1})        (pure elementwise add chain)
    out_t = sigmoid(10 * s * g_t)    (strided ACT sigmoid, scale = +-10)
    hidden = s * g_T
Each (pixel, component) is an independent scalar recurrence, so the batch
axis shards freely across the 8 cores.
"""

import numpy as np

T = 128          # sequence length
B = 262144       # total pixels
NCORES = 8
BC = B // NCORES  # 32768 pixels per core
P = 128           # SBUF partitions
F = BC * 2 // P   # 512 floats per partition per time step
TB = 8            # time-block (steps per DMA block)
NBLK = T // TB

_NC_CACHE = {}


def _build_nc(scale_e: float, scale_o: float):
    import concourse.bass as bass  # noqa: F401
    import concourse.tile as tile
    from concourse import bacc, mybir

    f32 = mybir.dt.float32
    AF = mybir.ActivationFunctionType

    nc = bacc.Bacc(None, target_bir_lowering=False, debug=False)
    x = nc.dram_tensor("x", [T, P * F], f32, kind="ExternalInput")
    h0 = nc.dram_tensor("h0", [P, F], f32, kind="ExternalInput")
    sgn = nc.dram_tensor("sgn", [P, F], f32, kind="ExternalInput")
    out = nc.dram_tensor("out", [T, P * F], f32, kind="ExternalOutput")
    hid = nc.dram_tensor("hidden", [P, F], f32, kind="ExternalOutput")

    with tile.TileContext(nc) as tc:
        with (
            tc.tile_pool(name="xp", bufs=2) as xp,
            tc.tile_pool(name="gp", bufs=2) as gp,
            tc.tile_pool(name="yp", bufs=2) as yp,
            tc.tile_pool(name="up", bufs=2, space="PSUM") as up,
            tc.tile_pool(name="smp", bufs=1) as smp,
        ):
            sg = smp.tile([P, F], f32, tag="sg")
            nc.sync.dma_start(sg[:], sgn[:])
            ht = smp.tile([P, F], f32, tag="ht")
            nc.sync.dma_start(ht[:], h0[:])
            g0 = smp.tile([P, F], f32, tag="g0")
            nc.vector.tensor_mul(g0[:], ht[:], sg[:])

            gprev = g0[:]
            prev = None  # (gb, yb, t0) of previous block
            for b in range(NBLK):
                t0 = b * TB
                xb = xp.tile([P, TB * F], f32)
                nc.sync.dma_start(
                    xb[:].rearrange("p (t f) -> p t f", t=TB),
                    x[t0:t0 + TB, :].rearrange("t (p f) -> p t f", p=P),
                )
                gb = gp.tile([P, TB * F], f32)
                yb = yp.tile([P, TB * F], f32)
                for j in range(TB):
                    t = t0 + j
                    u = up.tile([P, F], f32)
                    nc.vector.tensor_add(u[:], xb[:, j * F:(j + 1) * F], gprev)
                    nc.scalar.activation(gb[:, j * F:(j + 1) * F], u[:], AF.Tanh)
                    gprev = gb[:, j * F:(j + 1) * F]
                    if t % 2 == 1:
                        # even-component sigmoid over window {t-1, t}
                        w0 = (j - 1) * F
                        nc.scalar.activation(
                            yb[:, w0:w0 + 2 * F:2],
                            gb[:, w0:w0 + 2 * F:2],
                            AF.Sigmoid, scale=scale_e,
                        )
                    elif j >= 2:
                        # odd-component sigmoid over window {t-2, t-1}
                        w0 = (j - 2) * F
                        nc.scalar.activation(
                            yb[:, w0 + 1:w0 + 2 * F:2],
                            gb[:, w0 + 1:w0 + 2 * F:2],
                            AF.Sigmoid, scale=scale_o,
                        )
                    elif b > 0 and j == 0:
                        # odd sigmoid for last window of the previous block,
                        # then its y-block is complete: stream it out.
                        pgb, pyb, pt0 = prev
                        w0 = (TB - 2) * F
                        nc.scalar.activation(
                            pyb[:, w0 + 1:w0 + 2 * F:2],
                            pgb[:, w0 + 1:w0 + 2 * F:2],
                            AF.Sigmoid, scale=scale_o,
                        )
                        nc.sync.dma_start(
                            out[pt0:pt0 + TB, :].rearrange("t (p f) -> p t f", p=P),
                            pyb[:].rearrange("p (t f) -> p t f", t=TB),
                        )
                prev = (gb, yb, t0)

            pgb, pyb, pt0 = prev
            w0 = (TB - 2) * F
            nc.scalar.activation(
                pyb[:, w0 + 1:w0 + 2 * F:2],
                pgb[:, w0 + 1:w0 + 2 * F:2],
                AF.Sigmoid, scale=scale_o,
            )
            nc.sync.dma_start(
                out[pt0:pt0 + TB, :].rearrange("t (p f) -> p t f", p=P),
                pyb[:].rearrange("p (t f) -> p t f", t=TB),
            )
            hidt = smp.tile([P, F], f32, tag="hidt")
            nc.vector.tensor_mul(hidt[:], gprev, sg[:])
            nc.sync.dma_start(hid[:], hidt[:])

    nc.compile()
    return nc


def _get_nc(scale_e: float, scale_o: float):
    key = (scale_e, scale_o)
    if key not in _NC_CACHE:
        _NC_CACHE[key] = _build_nc(scale_e, scale_o)
    return _NC_CACHE[key]


def _make_in_maps(x, h, s0, s1):
    sgn_row = np.tile(np.array([s0, s1], dtype=np.float32), F // 2)
    sgn_np = np.ascontiguousarray(np.broadcast_to(sgn_row, (P, F)))
    in_maps = []
    for c in range(NCORES):
        xc = np.ascontiguousarray(x[:, c * BC:(c + 1) * BC, :]).reshape(T, P * F)
        hc = np.ascontiguousarray(h[0, c * BC:(c + 1) * BC, :]).reshape(P, F)
        in_maps.append({"x": xc, "h0": hc, "sgn": sgn_np})
    return in_maps


def _run(nc, in_maps, trace=False):
    from concourse.bass_utils import run_bass_kernel_spmd
    return run_bass_kernel_spmd(nc, in_maps, list(range(NCORES)), trace=trace)


def _assemble(results):
    out = np.concatenate(
        [results[c]["out"].reshape(T, BC, 2) for c in range(NCORES)], axis=1
    )
    hidden = np.concatenate(
        [results[c]["hidden"].reshape(BC, 2) for c in range(NCORES)], axis=0
    )[None]
    return out, hidden


def _numpy_ref(x, h, W_ih, W_hh, b_ih, b_hh):
    hp = h[0].astype(np.float32)
    bias = (b_ih + b_hh).astype(np.float32)
    out = np.empty_like(x)
    for t in range(x.shape[0]):
        hp = np.tanh(x[t] @ W_ih.T + hp @ W_hh.T + bias)
        out[t] = 1.0 / (1.0 + np.exp(-10.0 * hp))
    return out, hp[None]


def kernel(x, h, W_ih, W_hh, b_ih, b_hh):
    x = np.ascontiguousarray(np.asarray(x, dtype=np.float32))
    h = np.ascontiguousarray(np.asarray(h, dtype=np.float32))
    W_ih = np.asarray(W_ih, dtype=np.float32)
    W_hh = np.asarray(W_hh, dtype=np.float32)
    b_ih = np.asarray(b_ih, dtype=np.float32)
    b_hh = np.asarray(b_hh, dtype=np.float32)

    fast = (
        x.shape == (T, B, 2)
        and h.shape == (1, B, 2)
        and np.array_equal(W_hh, np.eye(2, dtype=np.float32))
        and np.all(b_ih + b_hh == 0.0)
        and W_ih[0, 1] == 0.0 and W_ih[1, 0] == 0.0
        and abs(W_ih[0, 0]) == 1.0 and abs(W_ih[1, 1]) == 1.0
    )
    if not fast:
        return _numpy_ref(x, h, W_ih, W_hh, b_ih, b_hh)

    s0 = float(W_ih[0, 0])
    s1 = float(W_ih[1, 1])
    nc = _get_nc(10.0 * s0, 10.0 * s1)
    in_maps = _make_in_maps(x, h, s0, s1)
    results = _run(nc, in_maps).results
    return _assemble(results)


# revision 4
# speedup vs baseline: 1.1255x; 1.1255x over previous
"""Pixel-RNN control kernel for TRN2 (8 NeuronCores, batch-sharded).

Math: h_t = tanh(x_t @ W_ih^T + h_{t-1} @ W_hh^T + b), out_t = sigmoid(10*h_t).
With W_hh = I, b = 0, W_ih = diag(s), |s_c| = 1, substitute g = s*h:
    g_t = tanh(x_t + g_{t-1})        (pure elementwise add chain)
    out_t = sigmoid(10 * s * g_t)    (ACT sigmoid with per-partition scale)
    hidden = s * g_T

Layout per core: component 0 lives in partitions 0-63, component 1 in
64-127, so the +-10 sigmoid scale is a per-partition [P,1] vector and
every step is one contiguous FD=512 instruction. ACT queue alternates
tanh/sigmoid strictly, hiding the DVE add latency each step.
"""

import numpy as np

T = 128          # sequence length
B = 262144       # total pixels
NCORES = 8
BC = B // NCORES  # 32768 pixels per core
P = 128           # SBUF partitions
Q = 64            # pixel-groups per component (P = 2 * Q)
F = BC // Q       # 512 floats per partition per time step
TB = 8            # time-block (steps per x/y tile)
HB = TB // 2      # half-block (steps per DMA)
NBLK = T // TB

_NC_CACHE = {}


def _build_nc(scale_e: float, scale_o: float):
    import concourse.bass as bass  # noqa: F401
    import concourse.tile as tile
    from concourse import bacc, mybir

    f32 = mybir.dt.float32
    AF = mybir.ActivationFunctionType

    nc = bacc.Bacc(None, target_bir_lowering=False, debug=False)
    x = nc.dram_tensor("x", [P, T * F], f32, kind="ExternalInput")
    h0 = nc.dram_tensor("h0", [P, F], f32, kind="ExternalInput")
    sgn = nc.dram_tensor("sgn", [P, 1], f32, kind="ExternalInput")
    scl = nc.dram_tensor("scl", [P, 1], f32, kind="ExternalInput")
    out = nc.dram_tensor("out", [P, T * F], f32, kind="ExternalOutput")
    hid = nc.dram_tensor("hidden", [P, F], f32, kind="ExternalOutput")

    HF = HB * F

    with tile.TileContext(nc) as tc:
        with (
            tc.tile_pool(name="xp", bufs=2) as xp,
            tc.tile_pool(name="gp", bufs=3) as gp,
            tc.tile_pool(name="yp", bufs=2) as yp,
            tc.tile_pool(name="up", bufs=2, space="PSUM") as up,
            tc.tile_pool(name="smp", bufs=1) as smp,
        ):
            sg = smp.tile([P, 1], f32, tag="sg")
            nc.sync.dma_start(sg[:], sgn[:])
            sc = smp.tile([P, 1], f32, tag="sc")
            nc.sync.dma_start(sc[:], scl[:])
            ht = smp.tile([P, F], f32, tag="ht")
            nc.sync.dma_start(ht[:], h0[:])
            # warm both ACT function tables during the DMA ramp
            warm = smp.tile([P, 1], f32, tag="warm")
            nc.scalar.activation(warm[:], sg[:], AF.Tanh)
            nc.scalar.activation(warm[:], sg[:], AF.Sigmoid)
            g0 = smp.tile([P, F], f32, tag="g0")
            nc.vector.tensor_scalar_mul(g0[:], ht[:], sg[:])

            gprev = g0[:]
            for b in range(NBLK):
                t0 = b * TB
                xb = xp.tile([P, TB * F], f32)
                nc.sync.dma_start(xb[:, :HF], x[:, t0 * F:t0 * F + HF])
                nc.sync.dma_start(
                    xb[:, HF:], x[:, t0 * F + HF:(t0 + TB) * F]
                )
                yb = yp.tile([P, TB * F], f32)
                for j in range(TB):
                    u = up.tile([P, F], f32)
                    nc.vector.tensor_add(u[:], xb[:, j * F:(j + 1) * F], gprev)
                    g = gp.tile([P, F], f32)
                    nc.scalar.activation(g[:], u[:], AF.Tanh)
                    nc.scalar.activation(
                        yb[:, j * F:(j + 1) * F], g[:], AF.Sigmoid,
                        scale=sc[:],
                    )
                    gprev = g[:]
                    if j == HB - 1:
                        nc.sync.dma_start(
                            out[:, t0 * F:t0 * F + HF], yb[:, :HF]
                        )
                nc.sync.dma_start(
                    out[:, t0 * F + HF:(t0 + TB) * F], yb[:, HF:]
                )

            hidt = smp.tile([P, F], f32, tag="hidt")
            nc.vector.tensor_scalar_mul(hidt[:], gprev, sg[:])
            nc.sync.dma_start(hid[:], hidt[:])

    nc.compile()
    return nc


def _get_nc(scale_e: float, scale_o: float):
    key = (scale_e, scale_o)
    if key not in _NC_CACHE:
        _NC_CACHE[key] = _build_nc(scale_e, scale_o)
    return _NC_CACHE[key]


def _make_in_maps(x, h, s0, s1):
    # partition p = comp*64 + q holds pixels [q*512, (q+1)*512) of its comp
    xt = np.ascontiguousarray(
        x.reshape(T, NCORES, Q, F, 2).transpose(1, 4, 2, 0, 3)
    ).reshape(NCORES, P, T * F)
    hp = np.ascontiguousarray(
        h[0].reshape(NCORES, Q, F, 2).transpose(0, 3, 1, 2)
    ).reshape(NCORES, P, F)
    sgn_np = np.repeat(
        np.array([s0, s1], dtype=np.float32), Q
    ).reshape(P, 1)
    scl_np = np.ascontiguousarray(10.0 * sgn_np)
    in_maps = []
    for c in range(NCORES):
        in_maps.append({
            "x": xt[c], "h0": hp[c], "sgn": sgn_np, "scl": scl_np,
        })
    return in_maps


def _run(nc, in_maps, trace=False):
    from concourse.bass_utils import run_bass_kernel_spmd
    return run_bass_kernel_spmd(nc, in_maps, list(range(NCORES)), trace=trace)


def _assemble(results):
    out = np.concatenate(
        [
            results[c]["out"].reshape(2, Q, T, F).transpose(2, 1, 3, 0)
            .reshape(T, BC, 2)
            for c in range(NCORES)
        ],
        axis=1,
    )
    hidden = np.concatenate(
        [
            results[c]["hidden"].reshape(2, Q, F).transpose(1, 2, 0)
            .reshape(BC, 2)
            for c in range(NCORES)
        ],
        axis=0,
    )[None]
    return np.ascontiguousarray(out), np.ascontiguousarray(hidden)


def _numpy_ref(x, h, W_ih, W_hh, b_ih, b_hh):
    hp = h[0].astype(np.float32)
    bias = (b_ih + b_hh).astype(np.float32)
    out = np.empty_like(x)
    for t in range(x.shape[0]):
        hp = np.tanh(x[t] @ W_ih.T + hp @ W_hh.T + bias)
        out[t] = 1.0 / (1.0 + np.exp(-10.0 * hp))
    return out, hp[None]


def kernel(x, h, W_ih, W_hh, b_ih, b_hh):
    x = np.ascontiguousarray(np.asarray(x, dtype=np.float32))
    h = np.ascontiguousarray(np.asarray(h, dtype=np.float32))
    W_ih = np.asarray(W_ih, dtype=np.float32)
    W_hh = np.asarray(W_hh, dtype=np.float32)
    b_ih = np.asarray(b_ih, dtype=np.float32)
    b_hh = np.asarray(b_hh, dtype=np.float32)

    fast = (
        x.shape == (T, B, 2)
        and h.shape == (1, B, 2)
        and np.array_equal(W_hh, np.eye(2, dtype=np.float32))
        and np.all(b_ih + b_hh == 0.0)
        and W_ih[0, 1] == 0.0 and W_ih[1, 0] == 0.0
        and abs(W_ih[0, 0]) == 1.0 and abs(W_ih[1, 1]) == 1.0
    )
    if not fast:
        return _numpy_ref(x, h, W_ih, W_hh, b_ih, b_hh)

    s0 = float(W_ih[0, 0])
    s1 = float(W_ih[1, 1])
    nc = _get_nc(10.0 * s0, 10.0 * s1)
    in_maps = _make_in_maps(x, h, s0, s1)
    results = _run(nc, in_maps).results
    return _assemble(results)
